# revision 33
# baseline (speedup 1.0000x reference)
"""Multi-head attention (B=2, S=2048, H=1024, 16 heads x 64) on 8 trn2 cores.

Sharding: core c handles batch b=c//4 and the 4 heads [4*(c%4) .. 4*(c%4)+3]
(tensor-parallel over the hd=256 column slice of Wq/Wk/Wv and the matching
row slice of Wo).  Each core computes a rank-256 partial of the output
projection for its batch; the host sums the 4 partials per batch and adds bo.

Device kernel (per core, bf16 matmuls with fp32 PSUM accumulate):
  QT[hd,s] = Wq_c^T X_b^T   (lhsT=Wq nat. layout, rhs=X^T prepped on host)
  KT[hd,s], V[s,hd] similarly.
  Per head pair (2 heads packed in the 128-partition dim):
    ST[k,q]  = KT_h^T QT_h           (K=64 row-packed pairs)
    PT       = exp(SCALE*ST + maskbias[k])   (ScalarE, mask folded into bias)
    OT/d together via plain M=128 matmuls whose stationary operand embeds a
    ones column next to V ([V_A|1|0] / [1|0|V_B]) so the softmax denominator
    accumulates in an otherwise-idle PSUM row of the same matmul.  Plain
    full-size matmuls keep the PE's background weight loads pipelined
    (tile-positioned pairs pay a ~175ns LDWEIGHTS stall per slot).
    Group end: two 1-row reciprocals, two K=1 replication matmuls broadcast
    1/d across the 64 PSUM rows of each head, two DVE mults normalize.
  Y_partial[s,H] = OT_norm^T Wo_c   (streamed out per 128-row tile, bf16)

Scheduling: one flat software pipeline paced by ScalarE's exp.  Projection
groups (QT/KT/V) and output-projection tiles are emitted as backlog items
drained between attention iterations, so the PE FIFO never head-of-line
blocks and startup/tail overlap the steady state.
"""
import sys

sys.path.insert(0, "/opt/trn_rl_repo")

import numpy as np
import ml_dtypes
from contextlib import ExitStack

B, S, H = 2, 2048, 1024
NH, HD = 16, 64
SCALE = 1.0 / float(np.sqrt(HD))
HPC = 4          # heads per core
HDC = HPC * HD   # 256 per-core head-dim slice
P = 128
KO = H // P      # 8 contraction tiles for the projections
ST_TILES = S // P    # 16
NQ = S // 512        # 4 q-chunks of 512
M2 = HDC // P        # 2 partition-tiles of the per-core head dim

_BUILT = {}


def _build(dt_name="bfloat16", debug_dump=False):
    import concourse.bacc as bacc
    import concourse.mybir as mybir
    import concourse.tile as tile

    DT = getattr(mybir.dt, dt_name)
    F32 = mybir.dt.float32

    nc = bacc.Bacc("TRN2", target_bir_lowering=False, debug=False)

    # all inputs pre-rearranged on host so DMAs are per-partition contiguous
    xt_d = nc.dram_tensor("xt", [NQ, P, KO, 512], DT, kind="ExternalInput").ap()
    wq_d = nc.dram_tensor("wq", [P, KO, HDC], DT, kind="ExternalInput").ap()
    wk_d = nc.dram_tensor("wk", [P, KO, HDC], DT, kind="ExternalInput").ap()
    wv_d = nc.dram_tensor("wv", [P, KO, HDC], DT, kind="ExternalInput").ap()
    wo_d = nc.dram_tensor("wo", [P, M2, H], DT, kind="ExternalInput").ap()
    bqt_d = nc.dram_tensor("bqt", [P, M2], F32, kind="ExternalInput").ap()
    bkt_d = nc.dram_tensor("bkt", [P, M2], F32, kind="ExternalInput").ap()
    bvr_d = nc.dram_tensor("bvr", [P, M2, 2, HD], F32, kind="ExternalInput").ap()
    mb_d = nc.dram_tensor("mb", [P, ST_TILES], F32, kind="ExternalInput").ap()
    y_d = nc.dram_tensor("y", [S, H], DT, kind="ExternalOutput").ap()

    with tile.TileContext(nc) as tc, ExitStack() as ctx:
        consts = ctx.enter_context(tc.tile_pool(name="consts", bufs=1))
        qkv = ctx.enter_context(tc.tile_pool(name="qkv", bufs=1))
        # deep pt pool: exp's buffer-reuse guard (WAR on the PV readers) must
        # tolerate the PV chain lagging ~3 iterations behind after each
        # group-boundary PSUM handoff
        pt_pool = ctx.enter_context(tc.tile_pool(name="pt", bufs=8))
        sm_pool = ctx.enter_context(tc.tile_pool(name="sm", bufs=2))
        y_pool = ctx.enter_context(tc.tile_pool(name="ysb", bufs=4))
        ps_proj = ctx.enter_context(tc.tile_pool(name="ps_proj", bufs=2, space="PSUM"))
        ps_st = ctx.enter_context(tc.tile_pool(name="ps_st", bufs=2, space="PSUM"))
        # one buffer holding both heads' O/denominator banks; the next group's
        # first PV waits on this group's normalization reads (PV bursts to
        # catch up afterwards)
        ps_ot = ctx.enter_context(tc.tile_pool(name="ps_ot", bufs=1, space="PSUM"))

        # ---- engine warmup during the input-DMA window ----
        # dummy matmuls trip the PE HAM clock-gate (3.4us busy window ->
        # 2.4GHz) and keep it warm until the input DMAs land (~15us); a dummy
        # exp preloads the ScalarE Exp table.
        ones_sb = consts.tile([P, 64], DT)
        nc.vector.memset(ones_sb[:], 1.0)
        warm_sb = consts.tile([P, 512], DT)
        nc.vector.memset(warm_sb[:], 1.0)
        warm_out = consts.tile([P, 64], DT)
        warm_ps = ps_proj.tile([P, 512], F32, tag="ps", name="warm_ps")
        for _ in range(28):
            nc.tensor.matmul(warm_ps[:], lhsT=warm_sb[:, 0:128], rhs=warm_sb[:],
                             start=True, stop=True)
        nc.scalar.activation(warm_out[:], ones_sb[:],
                             mybir.ActivationFunctionType.Exp,
                             bias=0.0, scale=1.0)

        # ---- input DMAs, balanced over three queues so the first-needed
        # tensors (wk, bkt, xt chunk 0) land as early as possible ----
        wk_sb = consts.tile([P, KO, HDC], DT)
        nc.scalar.dma_start(wk_sb[:], wk_d)
        wq_sb = consts.tile([P, KO, HDC], DT)
        nc.gpsimd.dma_start(wq_sb[:], wq_d)

        bqt_sb = consts.tile([P, M2], F32)
        nc.gpsimd.dma_start(bqt_sb[:], bqt_d)
        bkt_sb = consts.tile([P, M2], F32)
        nc.gpsimd.dma_start(bkt_sb[:], bkt_d)
        bvr_sb = consts.tile([P, M2, 2, HD], F32)
        nc.gpsimd.dma_start(bvr_sb[:], bvr_d)
        mb_sb = consts.tile([P, ST_TILES], F32)
        nc.scalar.dma_start(mb_sb[:], mb_d)
        wv_sb = consts.tile([P, KO, HDC], DT)
        nc.gpsimd.dma_start(wv_sb[:], wv_d)

        xt_sb = consts.tile([P, KO, S], DT)
        xt_eng = [nc.sync, nc.sync, nc.sync, nc.scalar]
        for c in range(NQ):
            cs = slice(c * 512, (c + 1) * 512)
            xt_eng[c].dma_start(xt_sb[:, :, cs], xt_d[c])

        wo_sb = consts.tile([P, M2, H], DT)
        nc.gpsimd.dma_start(wo_sb[:], wo_d)

        qt_sb = qkv.tile([P, M2, S], DT)
        kt_sb = qkv.tile([P, M2, S], DT)
        ot_sb = qkv.tile([P, M2, S], DT)
        # PV stationary operands, one [128,128] block per head:
        #   head 2m   (A): [V_A (64) | ones (1) | zeros (63)] -> d_A in row 64
        #   head 2m+1 (B): [ones (1) | zeros (63) | V_B (64)] -> d_B in row 0,
        #                                                      O_B in rows 64:
        # Plain full-width weights keep the PE pipelined, and the ones column
        # accumulates the softmax denominator in a spare PSUM row for free.
        v2_sb = qkv.tile([P, ST_TILES, M2, 2, P], DT)
        nc.vector.memset(v2_sb[:], 0.0)
        nc.vector.memset(v2_sb[:, :, :, 0, HD:HD + 1], 1.0)
        nc.vector.memset(v2_sb[:, :, :, 1, 0:1], 1.0)

        # ---- projection group emitters ----
        # A spec describes one 8-matmul accumulation group; emitting two specs
        # interleaved lets each group's LDWEIGHTS prefetch under the other
        # group's matmul streaming.
        def _qk_ops(args):
            w_sb, b_sb, out_sb, m, q = args
            qs = slice(q * 512, (q + 1) * 512)
            ps = ps_proj.tile([P, 512], F32, tag="ps", name="ps_qk")

            def mm(ko, start, stop):
                nc.tensor.matmul(
                    ps[:],
                    lhsT=w_sb[:, ko, m * P:(m + 1) * P],
                    rhs=xt_sb[:, ko, qs],
                    start=start, stop=stop,
                )

            def finish():
                nc.vector.tensor_add(
                    out_sb[:, m, qs], ps[:],
                    b_sb[:, m:m + 1].to_broadcast((P, 512)),
                )

            return mm, finish

        def _v_ops(args):
            (st,) = args
            ps = ps_proj.tile([P, M2, 2, HD], F32, tag="ps", name="ps_v")

            def mm(ko, start, stop):
                nc.tensor.matmul(
                    ps[:],
                    lhsT=xt_sb[:, ko, st * P:(st + 1) * P],
                    rhs=wv_sb[:, ko, :],
                    start=start, stop=stop,
                )

            def finish():
                # scatter V into the per-head blocks of v2_sb (even heads at
                # block cols 0:64, odd heads at 64:128)
                nc.vector.tensor_add(
                    v2_sb[:, st, :, 0, 0:HD], ps[:, :, 0, :], bvr_sb[:, :, 0, :])
                nc.vector.tensor_add(
                    v2_sb[:, st, :, 1, HD:P], ps[:, :, 1, :], bvr_sb[:, :, 1, :])

            return mm, finish

        # groups are emitted in four 2-matmul quarters so backlog injections
        # between attention iterations stay small and don't delay the exp
        # pacer; the psum tile and finish() span all quarters.
        NPC = 4  # pieces per projection group
        _group_state = {}

        def emit_proj_half(kind, args, piece, gid=None):
            key = (kind, gid if gid is not None else id(args))
            if piece == 0:
                _group_state[key] = (_qk_ops if kind == "qk" else _v_ops)(args)
            mm, finish = _group_state[key]
            kq = KO // NPC
            for ko in range(piece * kq, (piece + 1) * kq):
                mm(ko, ko == 0, ko == KO - 1)
            if piece == NPC - 1:
                finish()
                del _group_state[key]

        def emit_proj(specs):
            for kind, args in specs:
                for piece in range(NPC):
                    emit_proj_half(kind, args, piece, gid=id(args))

        def proj_qk(w_sb, b_sb, out_sb, m, q):
            emit_proj([("qk", (w_sb, b_sb, out_sb, m, q))])

        def proj_v(st):
            emit_proj([("v", (st,))])

        def emit_y_tile(st, n, tail=False):
            ss = slice(st * P, (st + 1) * P)
            ns = slice(n * 512, (n + 1) * 512)
            k = (2 * st + n) % 2
            # in the tail there is no other PSUM traffic: widen the ring by
            # borrowing ps_st slots and split casts across Vector/Scalar
            if tail and k:
                yp = ps_st.tile([P, 512], F32, tag="stp", name="yp_t")
            else:
                yp = ps_proj.tile([P, 512], F32, tag="ps", name="yp")
            for m in range(M2):
                nc.tensor.matmul(
                    yp[:],
                    lhsT=ot_sb[:, m, ss], rhs=wo_sb[:, m, ns],
                    start=(m == 0), stop=(m == M2 - 1),
                )
            y_sb = y_pool.tile([P, 512], DT, name="y_sb")
            # split the PSUM->SBUF casts across Vector and Scalar so neither
            # queue's backlog head-of-line blocks the PE on yp slot reuse
            if k:
                nc.scalar.copy(y_sb[:], yp[:])
            else:
                nc.vector.tensor_copy(y_sb[:], yp[:])
            # alternate DMA queues so back-to-back y tiles don't serialize
            eng = nc.sync if k == 0 else nc.gpsimd
            eng.dma_start(y_d[ss, ns], y_sb[:])

        # ---- backlog of work drained through the pipeline ----
        # items: (deadline_iter, kind, args); kept sorted by deadline.
        backlog = []
        _seq_no = [0]

        def add(deadline, kind, *args):
            if kind in ("qk", "v"):
                gid = _seq_no[0]
                for piece in range(NPC):
                    backlog.append((deadline, _seq_no[0], kind, (args, gid), piece))
                    _seq_no[0] += 1
            else:
                backlog.append((deadline, _seq_no[0], kind, args, None))
                _seq_no[0] += 1

        def run_item(kind, args):
            if kind == "qk":
                w_sb, b_sb, out_sb, m, q = args
                proj_qk(w_sb, b_sb, out_sb, m, q)
            elif kind == "v":
                proj_v(args[0])
            elif kind == "y":
                emit_y_tile(*args)

        def drain(i, budget):
            """Emit backlog items in deadline order.  qk groups gate the
            exp pacer (ST matmuls read them) so they are deadline-forced
            with lookahead; v/y only gate the PV side, which recovers, so
            they drain at `budget` per iteration — except as a correctness
            backstop v is forced at dl <= i+1 (emission must precede the PV
            that reads it)."""
            backlog.sort(key=lambda t: (t[0], t[1]))
            n = 0
            while backlog:
                dl, _, kind, args, piece = backlog[0]
                if dl <= i + 4 or n < budget:
                    backlog.pop(0)
                    if kind == "y":
                        emit_y_tile(*args)
                    else:
                        real_args, gid = args
                        emit_proj_half(kind, real_args, piece, gid=gid)
                    n += 1
                else:
                    break

        # attention group order: q-outer spreads Y work across the pipeline
        seq = [(q, m, kt) for q in range(NQ) for m in range(M2)
               for kt in range(ST_TILES)]
        giter = {}  # (q, m) -> start iter
        for i, (q, m, kt) in enumerate(seq):
            if kt == 0:
                giter[(q, m)] = i

        # backlog deadlines
        for m in range(M2):
            for j in range(NQ):
                if (m, j) != (0, 0):
                    # KT chunk j needed by kt=4j of every group of this m
                    add(giter[(0, m)] + 4 * j if (m, j) != (0, 0) else 0,
                        "qk", wk_sb, bkt_sb, kt_sb, m, j)
                if (m, j) != (0, 0):
                    add(giter[(j, m)], "qk", wq_sb, bqt_sb, qt_sb, m, j)
        for st in range(2, ST_TILES):
            add(st, "v", st)

        # ---- flat attention pipeline ----
        def st_mms(q, m, kt):
            ks = slice(kt * P, (kt + 1) * P)
            qs = slice(q * 512, (q + 1) * 512)
            stp = ps_st.tile([P, 1024], F32, name="stp", tag="stp")
            nc.tensor.matmul(
                stp[:, 0:512],
                lhsT=kt_sb[0:64, m, ks], rhs=qt_sb[0:64, m, qs],
                start=True, stop=True,
            )
            nc.tensor.matmul(
                stp[:, 512:1024],
                lhsT=kt_sb[64:128, m, ks], rhs=qt_sb[64:128, m, qs],
                start=True, stop=True,
            )
            return stp

        oa_ps = ob_ps = None

        def emit_pv(pt_, q_, m_, kt_):
            nonlocal oa_ps, ob_ps
            if kt_ == 0:
                oa_ps = ps_ot.tile([P, 512], F32, tag="oa", name="oa_ps")
                ob_ps = ps_ot.tile([P, 512], F32, tag="ob", name="ob_ps")
            # O^T (+ denominator row) accumulation: plain [128,128] weights
            nc.tensor.matmul(
                oa_ps[:], lhsT=v2_sb[:, kt_, m_, 0, :], rhs=pt_[:, 0:512],
                start=(kt_ == 0), stop=(kt_ == ST_TILES - 1),
            )
            nc.tensor.matmul(
                ob_ps[:], lhsT=v2_sb[:, kt_, m_, 1, :], rhs=pt_[:, 512:1024],
                start=(kt_ == 0), stop=(kt_ == ST_TILES - 1),
            )

        def emit_norm(q_, m_, tail=False):
            # 1/d for head A sits in oa_ps row 64, head B in ob_ps row 0;
            # replicate each across 64 rows with a K=1 matmul, then normalize.
            rec32 = sm_pool.tile([P, 512], F32, tag="rec32", name="rec32")
            # NB: DVE ops misbehave with a non-zero base partition; run the
            # head-A reciprocal over [0:65] (row 64 = 1/d_A, rows 0:63
            # garbage) and let head B's overwrite row 0 after.
            nc.vector.reciprocal_approx_fast(rec32[0:65, :], oa_ps[0:65, :])
            nc.vector.reciprocal_approx_fast(rec32[0:1, :], ob_ps[0:1, :])
            rec = sm_pool.tile([P, 512], DT, tag="rec", name="rec")
            nc.vector.tensor_copy(rec[0:65, :], rec32[0:65, :])
            rep_ps = ps_proj.tile([P, 512], F32, tag="ps", name="rep_ps")
            nc.tensor.matmul(
                rep_ps[0:64, :], lhsT=ones_sb[64:65, 0:64],
                rhs=rec[64:65, :], start=True, stop=True,
                tile_position=(64, 0),
            )
            nc.tensor.matmul(
                rep_ps[64:128, :], lhsT=ones_sb[0:1, 0:64],
                rhs=rec[0:1, :], start=True, stop=True,
                tile_position=(0, 64),
            )
            rep_sb = sm_pool.tile([P, 512], F32, tag="rep", name="rep_sb")
            nc.vector.tensor_copy(rep_sb[:], rep_ps[:])
            halves = 2 if tail else 1
            for h in range(halves):
                w = 512 // halves
                cs = slice(q_ * 512 + h * w, q_ * 512 + (h + 1) * w)
                ws = slice(h * w, (h + 1) * w)
                nc.vector.tensor_mul(
                    ot_sb[0:64, m_, cs], oa_ps[0:64, ws], rep_sb[0:64, ws])
                nc.vector.tensor_mul(
                    ot_sb[64:128, m_, cs], ob_ps[64:128, ws],
                    rep_sb[64:128, ws])
                if tail:
                    for st in range(q_ * 4 + 2 * h, q_ * 4 + 2 * h + 2):
                        for n in range(2):
                            emit_y_tile(st, n, tail=True)

        # PV runs one iteration behind exp/ST: the group-end normalization
        # chain (DVE-latency-bound) is then emitted after the next ST pair,
        # so it never head-of-line blocks the exp pacer, and the next
        # group's first PV (which waits on this group's PSUM handoff) is
        # emitted after the chain, keeping FIFO order consistent with the
        # data flow.
        # ---- prefix: the minimum needed for ST[0], then the rest ----
        # KT m0 kpos 0:128 gates ST[0]; emit it narrow so the first exp
        # starts ~2us earlier, then QT q0, then ST[0], then the remainder.
        kps = ps_proj.tile([P, 128], F32, tag="ps", name="kps")
        for ko in range(KO):
            nc.tensor.matmul(kps[:], lhsT=wk_sb[:, ko, 0:P],
                             rhs=xt_sb[:, ko, 0:128],
                             start=(ko == 0), stop=(ko == KO - 1))
        nc.vector.tensor_add(
            kt_sb[:, 0, 0:128], kps[:], bkt_sb[:, 0:1].to_broadcast((P, 128)))
        emit_proj([("qk", (wq_sb, bqt_sb, qt_sb, 0, 0))])  # QT m0 q0
        stp_cur = st_mms(*seq[0])
        kps2 = ps_proj.tile([P, 384], F32, tag="ps", name="kps2")
        for ko in range(KO):
            nc.tensor.matmul(kps2[:], lhsT=wk_sb[:, ko, 0:P],
                             rhs=xt_sb[:, ko, 128:512],
                             start=(ko == 0), stop=(ko == KO - 1))
        nc.vector.tensor_add(
            kt_sb[:, 0, 128:512], kps2[:],
            bkt_sb[:, 0:1].to_broadcast((P, 384)))
        emit_proj([("v", (0,))])
        emit_proj([("v", (1,))])

        pv_pending = None
        for i, (q, m, kt) in enumerate(seq):
            pt = pt_pool.tile([P, 1024], DT, name="pt")
            nc.scalar.activation(
                pt[:], stp_cur[:],
                mybir.ActivationFunctionType.Exp,
                bias=mb_sb[:, kt:kt + 1],
                scale=SCALE,
            )
            if i + 1 < len(seq):
                stp_next = st_mms(*seq[i + 1])
            if pv_pending is not None:
                pq, pm, pkt, ppt = pv_pending
                emit_pv(ppt, pq, pm, pkt)
                if pkt == ST_TILES - 1:
                    emit_norm(pq, pm)
                    if pm == M2 - 1:
                        idx = 0
                        for st in range(pq * 4, pq * 4 + 4):
                            for n in range(2):
                                add(i + 1 + 2 * idx, "y", st, n)
                                idx += 1
            pv_pending = (q, m, kt, pt)
            drain(i, 2)
            stp_cur = stp_next
        pq, pm, pkt, ppt = pv_pending
        emit_pv(ppt, pq, pm, pkt)
        emit_norm(pq, pm, tail=True)
        drain(10 ** 9, 10 ** 9)
        if debug_dump:
            dq = nc.dram_tensor("dbg_qt", [P, M2, S], DT, kind="ExternalOutput").ap()
            dk = nc.dram_tensor("dbg_kt", [P, M2, S], DT, kind="ExternalOutput").ap()
            dv = nc.dram_tensor("dbg_v2", [P, ST_TILES, M2, 2, P], DT,
                                kind="ExternalOutput").ap()
            do = nc.dram_tensor("dbg_ot", [P, M2, S], DT, kind="ExternalOutput").ap()
            nc.sync.dma_start(dq, qt_sb[:])
            nc.sync.dma_start(dk, kt_sb[:])
            nc.sync.dma_start(dv, v2_sb[:])
            nc.sync.dma_start(do, ot_sb[:])

    nc.compile()
    return nc


def _get_built(dt_name="bfloat16"):
    if dt_name not in _BUILT:
        _BUILT[dt_name] = _build(dt_name)
    return _BUILT[dt_name]


def _prep_core_inputs(c, hidden_states, attention_mask, Wq, bq, Wk, bk, Wv, bv, Wo, bo,
                      np_dt):
    b, g = c // 4, c % 4
    hs = slice(g * HDC, (g + 1) * HDC)
    xtT = hidden_states[b].T.astype(np_dt)          # [H, S]
    # xt[c, p, ko, s'] = X^T[ko*128+p, c*512+s']
    xt = np.ascontiguousarray(
        xtT.reshape(KO, P, NQ, 512).transpose(2, 1, 0, 3))

    def wqkv(W):  # [H, HDC] -> [P, KO, HDC]
        return np.ascontiguousarray(
            W[:, hs].astype(np_dt).reshape(KO, P, HDC).transpose(1, 0, 2))

    mb = np.where(attention_mask[b] == 0, np.float32(-30000.0), np.float32(0.0))
    return {
        "xt": xt,
        "wq": wqkv(Wq),
        "wk": wqkv(Wk),
        "wv": wqkv(Wv),
        "wo": np.ascontiguousarray(
            Wo[hs, :].astype(np_dt).reshape(M2, P, H).transpose(1, 0, 2)),
        "bqt": np.ascontiguousarray(bq[hs].reshape(M2, P).T).astype(np.float32),
        "bkt": np.ascontiguousarray(bk[hs].reshape(M2, P).T).astype(np.float32),
        "bvr": np.tile(bv[hs].astype(np.float32), (P, 1)).reshape(P, M2, 2, HD),
        "mb": np.ascontiguousarray(mb.astype(np.float32).reshape(ST_TILES, P).T),
    }


def kernel(hidden_states, attention_mask, Wq, bq, Wk, bk, Wv, bv, Wo, bo,
           _trace=False, _trace_kwargs=None):
    from concourse.bass_utils import run_bass_kernel_spmd

    hidden_states = np.asarray(hidden_states, np.float32)
    attention_mask = np.asarray(attention_mask)
    Wq, bq = np.asarray(Wq, np.float32), np.asarray(bq, np.float32)
    Wk, bk = np.asarray(Wk, np.float32), np.asarray(bk, np.float32)
    Wv, bv = np.asarray(Wv, np.float32), np.asarray(bv, np.float32)
    Wo, bo = np.asarray(Wo, np.float32), np.asarray(bo, np.float32)

    nc = _get_built()
    np_dt = ml_dtypes.bfloat16
    in_maps = [
        _prep_core_inputs(c, hidden_states, attention_mask,
                          Wq, bq, Wk, bk, Wv, bv, Wo, bo, np_dt)
        for c in range(8)
    ]
    kwargs = {}
    if _trace:
        kwargs["trace"] = True
        if _trace_kwargs:
            kwargs.update(_trace_kwargs)
    res = run_bass_kernel_spmd(nc, in_maps, core_ids=list(range(8)), **kwargs)
    out = np.empty((B, S, H), np.float32)
    for b in range(B):
        acc = res.results[4 * b]["y"].astype(np.float32)
        for c in range(4 * b + 1, 4 * b + 4):
            acc = acc + res.results[c]["y"].astype(np.float32)
        out[b] = acc + bo[None, :]
    if _trace:
        return out, res
    return out

